# revision 36
# baseline (speedup 1.0000x reference)
"""Trainium2 Bass kernel: single-head attention (B=4, S=4096, E=1024, D=64).

Distribution (8 NeuronCores): data-parallel over batch x query-halves.
Core c handles batch b = c//2 and query rows [h*2048, (h+1)*2048), h = c%2.
Each core computes K/V over the full sequence of its batch element
(weights replicated), so no collectives are needed.

Host-side layout prep (no FLOPs): x[b] is passed E-major (transposed) and in
bf16 so the E-contraction projection matmuls can DMA [128e, s] tiles
contiguously at half the HBM traffic, with the core's own query half permuted
to the first 2048 key columns (attention is permutation-invariant over keys).
The three projection weights are packed host-side into one bf16 tensor
[128p, 8e, 192] = [Wk | Wv | Wq] per e-chunk.

Device pipeline per core:
  1. QKV projections per 512-token s-group: K and V packed in ONE matmul per
     e-chunk (PSUM rows 0:64 = K^T, rows 64:128 = V^T) into a dedicated
     accumulator bank; Q accumulates in bank 0 of a rotating tile for the
     first 4 s-groups.  K^T/Q^T are copied to SBUF on 64 partitions.  V^T
     is transposed to natural [k, d] layout via PE transposes into bank 1
     of that rotating tile and augmented with a ones column (fused softmax
     denominator).
  2. Attention waves (3 key chunks x 512 queries): scoresT = K_chunk @ QT
     (64-partition contraction), exp via ACT with the 1/sqrt(64)
     scale folded into the activation pre-scale, then PV accumulation
     out_aug^T = V_aug.T @ exp(scoresT); row 64 of the accumulator is the
     softmax denominator.  The exp stream on ACT is the critical resource
     (~64us/core), so waves are software-pipelined INTO the projection
     stream: query-group qg only needs Q of s-group qg and K chunks up to
     its wave, so qg0's waves interleave with s-groups 1..7, then qg1/qg2
     and qg3/qg2 run pairwise — the ACT exp stream starts early and never
     drains between query groups.  Two time-shared accumulator banks
     (accA: pv0 then pv2; accB: stage-B KV then pv1 then pv3) make the
     pairwise interleave legal, and the per-group normalize is split so
     its PE transposes are deferred two waves behind the PSUM->SBUF pad
     copy and never block fresher score matmuls in the in-order PE queue.
  3. Finalize per query group: PE transpose of the [65, q] accumulator back
     to [q, 65] (into the accumulator's own bank, which is dead after the
     pad copy), DVE reciprocal + multiply to normalize, DMA out.

Softmax max-subtraction is skipped: scores are bounded (|s| < ~4) because
x ~ N(0,1) and W ~ U(-1/32, 1/32), so exp cannot overflow and softmax is
shift-invariant (result is mathematically identical).

The mask input is all-ones per the problem spec (fill=ones); a host check
falls back to a reference computation in the (never-expected) case it isn't.
"""

import numpy as np

B, S, E, D = 4, 4096, 1024, 64
N_CORES = 8
SQ = S // 2          # queries per core
P = 128
ECH = E // P         # 8 e-chunks of 128
SG = 512             # projection s-group width
NSG = S // SG        # 8
NQSG = SQ // SG      # first 4 s-groups hold this core's queries
QG = 512             # query group width for attention
NQG = SQ // QG       # 4
NKC = S // P         # 32 key chunks
NCH = SG // P        # key chunks per s-group (4)
WAVE = 3             # key chunks per exp wave (PSUM bank budget)

_progs = {}
LAST_RESULT = None


def _build(reps=1):
    """Build the Bass program. reps>1 repeats the whole kernel body inside
    one NEFF (same output) — used only for amplified HW timing in bench.py."""
    if reps in _progs:
        return _progs[reps]

    from collections import deque
    from contextlib import ExitStack

    import concourse.bacc as bacc
    import concourse.mybir as mybir
    import concourse.tile as tile
    from concourse.masks import make_identity

    f32 = mybir.dt.float32
    f32r = mybir.dt.float32r
    bf16 = mybir.dt.bfloat16
    Exp = mybir.ActivationFunctionType.Exp

    nc = bacc.Bacc("TRN2", target_bir_lowering=False)
    xt = nc.dram_tensor("xt", [E, S], bf16, kind="ExternalInput")
    wall = nc.dram_tensor("wall", [P, ECH, 3 * D], bf16, kind="ExternalInput")
    out = nc.dram_tensor("out", [SQ, D], f32, kind="ExternalOutput")

    xt_t = xt.rearrange("(c p) s -> p c s", p=P)            # [128, 8, S]
    out_t = out.rearrange("(g t p) d -> g p t d", p=P, t=QG // P)

    with tile.TileContext(nc) as tc:
      for _rep in range(reps):
       with ExitStack() as ctx:
        singles = ctx.enter_context(tc.tile_pool(name="singles", bufs=1))
        xpool = ctx.enter_context(tc.tile_pool(name="xstream", bufs=4))
        vtpool = ctx.enter_context(tc.tile_pool(name="vtsb", bufs=2))
        expool = ctx.enter_context(tc.tile_pool(name="expt", bufs=3))
        padpool = ctx.enter_context(tc.tile_pool(name="pad", bufs=2))
        opool = ctx.enter_context(tc.tile_pool(name="osb", bufs=2))
        rpool = ctx.enter_context(tc.tile_pool(name="rsb", bufs=2))
        # PSUM budget (8 banks): bigps 2 bufs x 3 banks + accA 1 + accB 1.
        # Long-lived accumulations get their own banks, time-disjoint per
        # pool: accA holds pv(qg0) then pv(qg2); accB holds the stage-B KV
        # accumulator per s-group, then pv(qg1), then pv(qg3).  The "big"
        # rotation only carries tiles with fast (DVE-copy) readers — score
        # tiles awaiting exp, and the Q/V-transpose projection tile — so
        # the PE never serializes on a slow reader two allocs back.
        bigps = ctx.enter_context(tc.tile_pool(name="bigps", bufs=2, space="PSUM"))
        accA = ctx.enter_context(tc.tile_pool(name="accA", bufs=1, space="PSUM"))
        accB = ctx.enter_context(tc.tile_pool(name="accB", bufs=1, space="PSUM"))

        # --- constants / persistent SBUF ---
        w_sb = singles.tile([P, ECH, 3 * D], bf16)
        nc.sync.dma_start(w_sb[:, 0, :], wall[:, 0, :])
        nc.sync.dma_start(w_sb[:, 1:ECH, :], wall[:, 1:ECH, :])
        ident = singles.tile([P, P], f32)
        make_identity(nc, ident)
        # f32r tiles can't be memset directly (ISA check); fill the ones
        # column via a DVE broadcast-copy from an f32 constant.
        oc = singles.tile([P, 1], f32)
        nc.gpsimd.memset(oc, 1.0)
        # K^T/Q^T on 64 partitions (the d contraction); trailing dims
        # [8, 64] mirror the PSUM projection tile so copies are
        # shape-compatible.
        kt_sb = singles.tile([D, NSG, SG // D, D], f32r)
        qt_sb = singles.tile([D, NQSG, SG // D, D], f32r)
        v_sb = singles.tile([P, NKC, D + 1], f32r)
        nc.vector.tensor_copy(v_sb[:, :, D], oc.to_broadcast([P, NKC]))

        # --- emission bodies (program order == engine issue order) ---
        def emit_proj(sg):
            s0, s1 = sg * SG, (sg + 1) * SG
            xt_tile = xpool.tile([P, ECH, SG], bf16, name="xt_tile")
            if sg == 0:
                nc.sync.dma_start(xt_tile[:, 0, :], xt_t[:, 0, s0:s1])
                nc.sync.dma_start(xt_tile[:, 1:4, :], xt_t[:, 1:4, s0:s1])
                nc.sync.dma_start(xt_tile[:, 4:8, :], xt_t[:, 4:8, s0:s1])
            else:
                nc.sync.dma_start(xt_tile, xt_t[:, :, s0:s1])
            kv = accB.tile([P, SG // D, D], f32, tag="acc", name="kv")
            qv = bigps.tile([P, WAVE, SG // D, D], f32, tag="big", name="qv")
            for c in range(ECH):
                nc.tensor.matmul(
                    kv, w_sb[:, c, 0:2 * D], xt_tile[:, c, :],
                    start=(c == 0), stop=(c == ECH - 1),
                )
            if sg < NQSG:
                for c in range(ECH):
                    nc.tensor.matmul(
                        qv[0:D, 0, :, :], w_sb[:, c, 2 * D:3 * D],
                        xt_tile[:, c, :],
                        start=(c == 0), stop=(c == ECH - 1),
                    )
            nc.vector.tensor_copy(kt_sb[:, sg], kv[0:D, :, :])
            if sg < NQSG:
                nc.vector.tensor_copy(qt_sb[:, sg], qv[0:D, 0, :, :])
            vt_sb = vtpool.tile([P, SG // D, D], f32, name="vt_sb")
            nc.vector.tensor_copy(vt_sb[D:P, :, :], kv[D:P, :, :])
            return sg, qv, vt_sb, xt_tile

        def emit_vtrans(proj_handle):
            sg, qv, vt_sb, _ = proj_handle
            for t in range(NCH):
                nc.tensor.transpose(
                    qv[:, 1, t, :],
                    vt_sb[D:P, 2 * t:2 * t + 2, :],
                    ident[D:P, D:P],
                )
            kc0 = sg * NCH
            nc.vector.tensor_copy(v_sb[:, kc0:kc0 + NCH, 0:D],
                                  qv[:, 1, 0:NCH, :])

        pv_tiles = {}
        fin_q = []                       # deferred finalize tails

        def finalize_pad(qg):
            # stage 1: move the accumulator to SBUF (DVE); the PE/DVE tail
            # is deferred so it never blocks fresher score matmuls in the
            # in-order PE queue
            pv_t = pv_tiles.pop(qg)
            pad = padpool.tile([D + 1, 4, P], f32, tag="pad", name="pad")
            nc.vector.tensor_copy(pad, pv_t[0:D + 1, :, :])
            fin_q.append([qg, pv_t, pad, 0])

        def finalize_rest(entry):
            qg, pv_t, pad, _ = entry
            # transpose back into the pv tile's own bank (dead after the
            # pad copy): pv_t[:, a, 0:65] holds [128q, 65] per slot
            for a in range(4):
                nc.tensor.transpose(
                    pv_t[:, a, 0:D + 1], pad[:, a, :],
                    ident[0:D + 1, 0:D + 1],
                )
            rr = rpool.tile([P, 4], f32, tag="rr", name="rr")
            nc.vector.reciprocal(rr, pv_t[:, :, D])
            ob = opool.tile([P, 4, D], f32, tag="ob", name="ob")
            nc.vector.tensor_mul(
                ob, pv_t[:, :, 0:D],
                rr[:, :, None].to_broadcast([P, 4, D])
            )
            nc.sync.dma_start(out_t[qg], ob)

        def tick_fin(force=False):
            # age deferred finalize tails; emit once 2 waves old (pad copy
            # has certainly landed by then)
            for entry in list(fin_q):
                entry[3] += 1
                if force or entry[3] >= 2:
                    finalize_rest(entry)
                    fin_q.remove(entry)

        prev = [None]                    # (qg, k0, nw, exp tile)

        def flush_pv(last=False):
            if prev[0] is None:
                return
            pqg, pk0, pnw, pet = prev[0]
            ppv = pv_tiles[pqg][0:D + 1, :, :]
            for w in range(pnw):
                kc = pk0 + w
                nc.tensor.matmul(
                    ppv, v_sb[:, kc, :], pet[:, w, :, :],
                    start=(kc == 0), stop=(kc == NKC - 1),
                )
            prev[0] = None
            if pk0 + pnw == NKC:         # that was pqg's final wave
                finalize_pad(pqg)
            if last:
                tick_fin(force=True)

        acc_by_qg = {0: accA, 1: accB, 2: accA, 3: accB}

        def emit_wave(qg, k0, nw):
            if qg not in pv_tiles:
                pv_tiles[qg] = acc_by_qg[qg].tile([P, 4, P], f32, tag="acc",
                                                  name="pv")
            qs = qt_sb[:, qg]
            sc = bigps.tile([P, WAVE, SG // D, D], f32, tag="big", name="sc")
            for w in range(nw):
                kc = k0 + w
                nc.tensor.matmul(
                    sc[:, w, :, :],
                    kt_sb[:, kc // NCH, 2 * (kc % NCH):2 * (kc % NCH) + 2, :],
                    qs, start=True, stop=True,
                )
            et = expool.tile([P, WAVE, SG // D, D], f32r, name="et")
            nc.scalar.activation(et[:, 0:nw, :, :], sc[:, 0:nw, :, :], Exp,
                                 scale=0.125)
            flush_pv()
            tick_fin()
            prev[0] = (qg, k0, nw, et)

        # --- driver: software-pipelined emission ---
        wave_sizes = [WAVE] * (NKC // WAVE)
        if NKC % WAVE:
            wave_sizes.append(NKC % WAVE)
        waves = []
        k0 = 0
        for nw in wave_sizes:
            waves.append((k0, nw))
            k0 += nw

        pend = {qg: deque(waves) for qg in range(NQG)}

        def can_emit(qg, sg):
            if not pend[qg] or qg > sg:
                return False
            wk0, wnw = pend[qg][0]
            return wk0 + wnw <= NCH * (sg + 1)

        for sg in range(NSG):
            handle = emit_proj(sg)
            did_trans = False
            while can_emit(0, sg):
                wk0, wnw = pend[0].popleft()
                emit_wave(0, wk0, wnw)
                if not did_trans:
                    emit_vtrans(handle)
                    did_trans = True
            if not did_trans:
                emit_vtrans(handle)
        while pend[1]:                   # qg1 + qg2 pairwise
            for qg in (1, 2):
                if pend[qg]:
                    wk0, wnw = pend[qg].popleft()
                    emit_wave(qg, wk0, wnw)
        while pend[2] or pend[3]:        # qg3 leads so its finalize overlaps
            for qg in (3, 2):
                if pend[qg]:
                    wk0, wnw = pend[qg].popleft()
                    emit_wave(qg, wk0, wnw)
        flush_pv(last=True)

    nc.compile()
    _progs[reps] = nc
    return nc


def _host_reference(x, Wq, Wk, Wv, mask):
    """Numpy fallback, only used if the mask is not all-ones (spec: it is)."""
    out = np.empty((B, S, D), np.float32)
    q = np.einsum("bse,de->bsd", x, Wq).astype(np.float32)
    k = np.einsum("bse,de->bsd", x, Wk).astype(np.float32)
    v = np.einsum("bse,de->bsd", x, Wv).astype(np.float32)
    scale = np.float32(1.0 / np.sqrt(D))
    for b in range(B):
        s = (q[b] @ k[b].T) * scale
        s = np.where(mask[b] == 0, -np.inf, s)
        s = s - s.max(axis=-1, keepdims=True)
        e = np.exp(s)
        a = e / e.sum(axis=-1, keepdims=True)
        out[b] = a @ v[b]
    return out


def kernel(x, Wq, Wk, Wv, mask, _trace=False):
    global LAST_RESULT
    import ml_dtypes

    bf16 = ml_dtypes.bfloat16

    x = np.ascontiguousarray(np.asarray(x), dtype=np.float32)
    Wq = np.ascontiguousarray(np.asarray(Wq), dtype=np.float32)
    Wk = np.ascontiguousarray(np.asarray(Wk), dtype=np.float32)
    Wv = np.ascontiguousarray(np.asarray(Wv), dtype=np.float32)
    mask = np.asarray(mask)

    if mask.min() == 0:
        return _host_reference(x, Wq, Wk, Wv, mask)

    from concourse.bass_utils import run_bass_kernel_spmd

    nc = _build()
    # packed weights [128p, 8e, 192] = [Wk | Wv | Wq] per e-chunk, bf16
    wcat = np.concatenate([Wk.T, Wv.T, Wq.T], axis=1)        # [E, 192]
    wall = np.ascontiguousarray(
        wcat.reshape(ECH, P, 3 * D).transpose(1, 0, 2)
    ).astype(bf16)
    in_maps = []
    for c in range(N_CORES):
        b, h = divmod(c, 2)
        xT = x[b].T.astype(bf16)                              # [E, S]
        if h == 0:
            xt_core = np.ascontiguousarray(xT)
        else:
            xt_core = np.ascontiguousarray(
                np.concatenate([xT[:, SQ:], xT[:, :SQ]], axis=1)
            )
        in_maps.append({"xt": xt_core, "wall": wall})

    res = run_bass_kernel_spmd(
        nc, in_maps, core_ids=list(range(N_CORES)), trace=_trace
    )
    LAST_RESULT = res

    out = np.empty((B, S, D), np.float32)
    for c in range(N_CORES):
        b, h = divmod(c, 2)
        out[b, h * SQ:(h + 1) * SQ] = res.results[c]["out"]
    return out
